# revision 22
# baseline (speedup 1.0000x reference)
"""Trainium2 Bass kernel for nn_DecoderA (neural BP / GNN message passing decoder).

Strategy: pure data parallel over batch (128 items -> 8 cores x 16 items).
Per core, items are processed in 4 groups of 4; each group's message state
lives in SBUF as 9 tiles of [128, 576] ((b,m)-rows x n) in fp16 for all 5 BP
iterations.  The state is stored with an off-mask sentinel baked in:
    Ms = M - 1024*2^{-t} * (1 - mask)
so the PE's  vr = Esel@Ahi + Esel@Alo - I@Ms  makes off-mask entries hugely
positive and tanh saturates them to exactly 1.0 in fp16 -- no mask multiply,
no clip, no DVE op between PE and ACT at all.  The sentinel decays by the
gate factor 0.5 each iteration (stays >> 1) and its effect on the posterior
column sums is corrected with host-precomputed per-(t,n) constants folded
into the xs tensor (device) and posts correction (host).

Per (group, iter), phase-split so the ACT engine loads each function table
once per (t, group-pair):
  PE    vr_j  = Esel@Ahi + Esel@Alo - I@Ms        (fp16 matmuls, fp32 PSUM)
  ACT   te_j  = tanh(0.5*vr_j)  [from PSUM]       (fp16; off-mask -> 1.0;
                |V|>15 saturates == clip in fp16)
  DVE   ps_j  = prefix-product scan of te_j       (fp16; last col = P_j)
  ACT   sq1_j = Square(te_j + P_j)                (bf16 out, fp32 internal)
  ACT   sq2_j = Square(-te_j + P_j)               (bf16 out)
  ACT   l12   = Ln(sq + 1e-38)  [one giant]       (fp16; = 2*ln|te +- P|)
  DVE   lq    = l1 - l2  = 4*atanh(P/te)          (fp16, pair-wide)
  DVE   lqc   = clip(lq, +-2C)                    (fp16)
  GPS   d     = lqc * Wg'  (Wg' = 0.5*gate*w_cv*H, fp16, from host)
  DVE   Ms    = Ms*(1-gate) + d                   (fp16 stt; decays sentinel)
  PE    post  = Esel^T @ Ms                       (fp16, accumulated)

Host does the cheap pre/post work (LLR normalization, sentinel correction,
pooling, sigmoid).  Two group streams are interleaved so every engine queue
holds independent work from both streams.
"""

import sys

import numpy as np

sys.path.insert(0, "/opt/trn_rl_repo")

import concourse.bacc as bacc  # noqa: E402
import concourse.tile as tile  # noqa: E402
from concourse import mybir  # noqa: E402
from concourse.bass_utils import run_bass_kernel_spmd  # noqa: E402

F32 = mybir.dt.float32
F16 = mybir.dt.float16
BF16 = mybir.dt.bfloat16
ALU = mybir.AluOpType
ACT = mybir.ActivationFunctionType

B = 128
MCHK = 288
NVAR = 576
KINFO = 288
T = 5
NCORES = 8
BL = B // NCORES          # 16 items per core
GI = 4                    # items per group
NG = BL // GI             # 4 groups
NT = GI * MCHK // 128     # 9 tiles of [128, NVAR] per group
HC = NVAR // 2            # 288, matmul N-chunk (<=512 per PSUM bank)
SENT = 1024.0             # off-mask sentinel magnitude at t=0 (pre-decay)

# lq is 2*(2*atanh(r)); clip at 2*C where C = 2*atanh(1-1e-6)
_CLIP2 = float(4.0 * np.arctanh(np.float64(np.float32(1.0 - 1e-6))))


def _build(gate: float):
    nc = bacc.Bacc("TRN2", target_bir_lowering=False, debug=False)

    wg_d = nc.dram_tensor("wg", [BL * MCHK, NVAR], F16, kind="ExternalInput").ap()
    ms_d = nc.dram_tensor("ms", [BL * MCHK, NVAR], F16, kind="ExternalInput").ap()
    xs_d = nc.dram_tensor("xs", [BL, T * NVAR], F32, kind="ExternalInput").ap()
    xh_d = nc.dram_tensor("xh", [BL, NVAR], F16, kind="ExternalInput").ap()
    esel_d = nc.dram_tensor("esel", [128, NT * GI], F16, kind="ExternalInput").ap()
    eselt_d = nc.dram_tensor("eselt", [GI, NT * 128], F16, kind="ExternalInput").ap()
    negi_d = nc.dram_tensor("negi", [128, 128], F16, kind="ExternalInput").ap()
    posts_d = nc.dram_tensor("posts", [BL, T * NVAR], F32, kind="ExternalOutput").ap()

    one_m_g = float(1.0 - gate)
    posts_v = posts_d.rearrange("b (t c n) -> b t c n", t=T, c=2)

    with tile.TileContext(nc) as tc:
        with (
            tc.tile_pool(name="consts", bufs=1) as consts,
            tc.tile_pool(name="wg", bufs=2) as wg_pool,
            tc.tile_pool(name="mstate", bufs=2) as m_pool,
            tc.tile_pool(name="te", bufs=2) as te_pool,
            tc.tile_pool(name="pscan", bufs=2) as ps_pool,
            tc.tile_pool(name="a12", bufs=1) as a12_pool,
            tc.tile_pool(name="l12", bufs=1) as l12_pool,
            tc.tile_pool(name="lq", bufs=2) as lq_pool,
            tc.tile_pool(name="acur", bufs=1) as a_pool,
            tc.tile_pool(name="psum_v", bufs=2, space="PSUM") as psv_pool,
            tc.tile_pool(name="psum_post", bufs=2, space="PSUM") as psp_pool,
        ):
            esel = consts.tile([128, NT, GI], F16)
            nc.sync.dma_start(out=esel, in_=esel_d.rearrange("p (j g) -> p j g", g=GI))
            eselt = consts.tile([GI, NT, 128], F16)
            nc.sync.dma_start(
                out=eselt, in_=eselt_d.rearrange("g (j p) -> g j p", p=128)
            )
            negi = consts.tile([128, 128], F16)
            nc.sync.dma_start(out=negi, in_=negi_d)
            b38 = consts.tile([128, 1], F32)
            nc.vector.memset(b38, 1e-38)
            xsall = consts.tile([128, T, 2, HC], F32)
            for g in range(NG):
                nc.sync.dma_start(
                    out=xsall[32 * g : 32 * g + GI],
                    in_=xs_d[g * GI : (g + 1) * GI].rearrange(
                        "b (t c n) -> b t c n", t=T, c=2
                    ),
                )

            # tile pairs for pair-wide tail ops: (0,1) (2,3) (4,5) (6,7) (8,)
            pairs = [(0, 1), (2, 3), (4, 5), (6, 7), (8,)]

            def load_group(g):
                st = {"g": g, "k": g % 2}
                wg_g = wg_pool.tile([128, NT, NVAR], F16)
                nc.sync.dma_start(
                    out=wg_g,
                    in_=wg_d[g * NT * 128 : (g + 1) * NT * 128, :].rearrange(
                        "(j p) n -> p j n", p=128
                    ),
                )
                m_g = m_pool.tile([128, NT, NVAR], F16, name="m")
                nc.sync.dma_start(
                    out=m_g,
                    in_=ms_d[g * NT * 128 : (g + 1) * NT * 128, :].rearrange(
                        "(j p) n -> p j n", p=128
                    ),
                )
                a16 = a_pool.tile([GI, 2, HC], F16, tag=f"a16{g % 2}", name="a16")
                nc.sync.dma_start(
                    out=a16,
                    in_=xh_d[g * GI : (g + 1) * GI].rearrange("b (c n) -> b c n", c=2),
                )
                st["wg"] = wg_g
                st["m"] = m_g
                st["a16"] = a16
                st["xs"] = xsall[32 * g : 32 * g + GI]
                return st

            def phase_ab(st, t):
                """PE broadcast matmuls; ACT tanh straight from PSUM."""
                k = st["k"]
                te_all = te_pool.tile([128, NT, NVAR], F16, tag=f"te{k}", name="te")
                st["te"] = te_all
                for j in range(NT):
                    v_ps = psv_pool.tile([128, 2, 512], F32)
                    for c in range(2):
                        nc.tensor.matmul(
                            v_ps[:, c, :HC], eselt[:, j], st["a16"][:, c],
                            start=True, stop=False,
                        )
                        nc.tensor.matmul(
                            v_ps[:, c, :HC], negi,
                            st["m"][:, j, c * HC : (c + 1) * HC],
                            start=False, stop=True,
                        )
                    nc.scalar.activation(
                        te_all[:, j].rearrange("p (c n) -> p c n", c=2),
                        v_ps[:, :, :HC], ACT.Tanh, scale=0.5,
                    )

            def phase_cd(st, t):
                """Scans + squares (ln-table), then one giant Ln."""
                k = st["k"]
                te_all = st["te"]
                a12 = a12_pool.tile([128, NT, 2, NVAR], BF16, tag="a12", name="a12")
                for pj in pairs:
                    ps = ps_pool.tile([128, 2, HC], F16, tag=f"ps{k}", name="ps")
                    for jj, j in enumerate(pj):
                        nc.vector.tensor_tensor_scan(
                            out=ps[:, jj], data0=te_all[:, j, :HC],
                            data1=te_all[:, j, HC:],
                            initial=1.0, op0=ALU.mult, op1=ALU.mult,
                        )
                    for jj, j in enumerate(pj):
                        p_t = ps[:, jj, HC - 1 : HC]
                        nc.scalar.activation(
                            a12[:, j, 0], te_all[:, j], ACT.Square, bias=p_t, scale=1.0
                        )
                        nc.scalar.activation(
                            a12[:, j, 1], te_all[:, j], ACT.Square, bias=p_t, scale=-1.0
                        )
                l12 = l12_pool.tile([128, NT, 2, NVAR], F16, tag=f"l12{k}", name="l12")
                nc.scalar.activation(l12, a12, ACT.Ln, bias=b38)
                st["l12"] = l12

            def phase_f(st, t):
                """Pair-wide tail: lq, clip, wg-mult, gate update, posterior."""
                k = st["k"]
                g = st["g"]
                l12 = st["l12"]
                m_g, wg_g = st["m"], st["wg"]
                post_ps = psp_pool.tile([GI, 2, 512], F32)
                for pj in pairs:
                    w = len(pj)
                    j0 = pj[0]
                    lqf = lq_pool.tile([128, 2, NVAR], F16, tag=f"lq{k}",
                                       name="lq")[:, :w]
                    nc.vector.tensor_tensor(
                        out=lqf, in0=l12[:, j0 : j0 + w, 0],
                        in1=l12[:, j0 : j0 + w, 1], op=ALU.subtract,
                    )
                    nc.vector.tensor_scalar(
                        out=lqf, in0=lqf, scalar1=_CLIP2, scalar2=-_CLIP2,
                        op0=ALU.min, op1=ALU.max,
                    )
                    nc.gpsimd.tensor_tensor(out=lqf, in0=lqf,
                                            in1=wg_g[:, j0 : j0 + w], op=ALU.mult)
                    nc.vector.scalar_tensor_tensor(
                        out=m_g[:, j0 : j0 + w], in0=m_g[:, j0 : j0 + w],
                        scalar=one_m_g, in1=lqf, op0=ALU.mult, op1=ALU.add,
                    )
                    for j in pj:
                        for c in range(2):
                            nc.tensor.matmul(
                                post_ps[:, c, :HC],
                                esel[:, j],
                                m_g[:, j, c * HC : (c + 1) * HC],
                                start=(j == 0),
                                stop=(j == NT - 1),
                            )
                # posts_raw[g, :, t, :] = post (host corrects sentinel, adds x_t)
                p32 = a_pool.tile([GI, 2, HC], F32, tag=f"p32{k}", name="p32")
                nc.vector.tensor_copy(p32, post_ps[:, :, :HC])
                nc.sync.dma_start(out=posts_v[g * GI : (g + 1) * GI, t], in_=p32)
                if t + 1 < T:
                    a16 = a_pool.tile([GI, 2, HC], F16, tag=f"a16{k}", name="a16")
                    nc.vector.tensor_add(a16, post_ps[:, :, :HC], st["xs"][:, t + 1])
                    st["a16"] = a16

            for gpair in ((0, 1), (2, 3)):
                sts = [load_group(g) for g in gpair]
                for t in range(T):
                    for st in sts:
                        phase_ab(st, t)
                    for st in sts:
                        phase_cd(st, t)
                    for st in sts:
                        phase_f(st, t)
    nc.compile()
    return nc


_CACHE = {}


def _get_nc(gate: float):
    key = round(gate, 12)
    if key not in _CACHE:
        _CACHE[key] = _build(gate)
    return _CACHE[key]


def _host_prep(inputs, H, sigma2, input_ponderation, w_cv, gate_logit):
    f32 = np.float32
    f16 = np.float16
    gate = float(1.0 / (1.0 + np.exp(-np.float64(gate_logit))))

    llrs = (f32(-4.0) * inputs / sigma2).astype(f32)
    norm_llrs = llrs / np.mean(np.abs(llrs), axis=-1, keepdims=True, dtype=f32)
    xs = (norm_llrs[:, None, :] * input_ponderation[None, :, :]).astype(f32)  # [B,T,N]
    xh0 = xs[:, 0].astype(f16)

    # sentinel corrections: posterior of iter t-1 is short by
    # SENT*2^{-t} per off-mask row of each column
    offc = (f32(MCHK) - H.sum(axis=1)).astype(f32)                # [B, N]
    xs_dev = xs.copy()
    for t in range(1, T):
        xs_dev[:, t] += f32(SENT * (2.0 ** (-t))) * offc
    post_corr = np.stack(
        [f32(SENT * (2.0 ** (-(t + 1)))) * offc for t in range(T)], axis=1
    )  # [B, T, N]

    # lq = 2*(2*atanh(r)); fold the 0.5 and the gate into the edge weights
    wgp = (f32(0.5 * gate) * w_cv[None, :, :] * H.astype(f32)).astype(f16)  # [B,M,N]
    msinit = (-f32(SENT) * (f32(1.0) - H.astype(f32))).astype(f16)

    # selector constants (same for every core)
    rows = np.arange(GI * MCHK)
    esel = np.zeros((128, NT, GI), f16)
    eselt = np.zeros((GI, NT, 128), f16)
    for j in range(NT):
        for p in range(128):
            k = int(rows[j * 128 + p] // MCHK)
            esel[p, j, k] = 1.0
            eselt[k, j, p] = 1.0
    negi = (-np.eye(128)).astype(f16)

    in_maps = []
    for c in range(NCORES):
        sl = slice(c * BL, (c + 1) * BL)
        in_maps.append(
            {
                "wg": np.ascontiguousarray(wgp[sl].reshape(BL * MCHK, NVAR)),
                "ms": np.ascontiguousarray(msinit[sl].reshape(BL * MCHK, NVAR)),
                "xs": np.ascontiguousarray(xs_dev[sl].reshape(BL, T * NVAR)),
                "xh": np.ascontiguousarray(xh0[sl]),
                "esel": np.ascontiguousarray(esel.reshape(128, NT * GI)),
                "eselt": np.ascontiguousarray(eselt.reshape(GI, NT * 128)),
                "negi": negi,
            }
        )
    return gate, norm_llrs, xs, post_corr, in_maps


def _host_post(posts_raw, xs, norm_llrs, post_corr, out_ponderation,
               skip_ponderation):
    f32 = np.float32
    posts = (posts_raw + post_corr + xs).astype(f32)  # un-poison + add x_t
    norm_out = posts / np.mean(np.abs(posts), axis=-1, keepdims=True, dtype=f32)
    pooled = np.mean(out_ponderation[None] * norm_out, axis=-2, dtype=f32)
    out = (pooled + skip_ponderation * norm_llrs).astype(f32)
    return (1.0 / (1.0 + np.exp(out[:, :KINFO], dtype=f32))).astype(f32)


def run(trace=False, **inputs):
    inputs = {k: np.asarray(v) for k, v in inputs.items()}
    gate, norm_llrs, xs, post_corr, in_maps = _host_prep(
        inputs["inputs"],
        inputs["H"],
        inputs["sigma2"],
        inputs["input_ponderation"],
        inputs["w_cv"],
        inputs["gate_logit"],
    )
    nc = _get_nc(gate)
    res = run_bass_kernel_spmd(
        nc, in_maps, core_ids=list(range(NCORES)), trace=trace
    )
    posts_raw = np.concatenate(
        [r["posts"].astype(np.float32).reshape(BL, T, NVAR) for r in res.results],
        axis=0,
    )
    out = _host_post(
        posts_raw, xs, norm_llrs, post_corr,
        inputs["out_ponderation"], inputs["skip_ponderation"],
    )
    return out, res


def kernel(**inputs) -> np.ndarray:
    out, _ = run(trace=False, **inputs)
    return out


# revision 23
# speedup vs baseline: 1.0003x; 1.0003x over previous
"""Trainium2 Bass kernel for nn_DecoderA (neural BP / GNN message passing decoder).

Strategy: pure data parallel over batch (128 items -> 8 cores x 16 items).
Per core, items are processed in 4 groups of 4; each group's message state
lives in SBUF as 9 tiles of [128, 576] ((b,m)-rows x n) in fp16 for all 5 BP
iterations.  The state is stored with an off-mask sentinel baked in:
    Ms = M - 1024*2^{-t} * (1 - mask)
so the PE's  vr = Esel@Ahi + Esel@Alo - I@Ms  makes off-mask entries hugely
positive and tanh saturates them to exactly 1.0 in fp16 -- no mask multiply,
no clip, no DVE op between PE and ACT at all.  The sentinel decays by the
gate factor 0.5 each iteration (stays >> 1) and its effect on the posterior
column sums is corrected with host-precomputed per-(t,n) constants folded
into the xs tensor (device) and posts correction (host).

Per (group, iter), phase-split so the ACT engine loads each function table
once per (t, group-pair):
  PE    vr_j  = Esel@Ahi + Esel@Alo - I@Ms        (fp16 matmuls, fp32 PSUM)
  ACT   te_j  = tanh(0.5*vr_j)  [from PSUM]       (fp16; off-mask -> 1.0;
                |V|>15 saturates == clip in fp16)
  DVE   ps_j  = prefix-product scan of te_j       (fp16; last col = P_j)
  ACT   sq1_j = Square(te_j + P_j)                (bf16 out, fp32 internal)
  ACT   sq2_j = Square(-te_j + P_j)               (bf16 out)
  ACT   l12   = Ln(sq + 1e-38)  [one giant]       (fp16; = 2*ln|te +- P|)
  DVE   lq    = l1 - l2  = 4*atanh(P/te)          (fp16, pair-wide)
  DVE   lqc   = clip(lq, +-2C)                    (fp16)
  GPS   d     = lqc * Wg'  (Wg' = 0.5*gate*w_cv*H, fp16, from host)
  DVE   Ms    = Ms*(1-gate) + d                   (fp16 stt; decays sentinel)
  PE    post  = Esel^T @ Ms                       (fp16, accumulated)

Host does the cheap pre/post work (LLR normalization, sentinel correction,
pooling, sigmoid).  Two group streams are interleaved so every engine queue
holds independent work from both streams.
"""

import sys

import numpy as np

sys.path.insert(0, "/opt/trn_rl_repo")

import concourse.bacc as bacc  # noqa: E402
import concourse.tile as tile  # noqa: E402
from concourse import mybir  # noqa: E402
from concourse.bass_utils import run_bass_kernel_spmd  # noqa: E402

F32 = mybir.dt.float32
F16 = mybir.dt.float16
BF16 = mybir.dt.bfloat16
ALU = mybir.AluOpType
ACT = mybir.ActivationFunctionType

B = 128
MCHK = 288
NVAR = 576
KINFO = 288
T = 5
NCORES = 8
BL = B // NCORES          # 16 items per core
GI = 4                    # items per group
NG = BL // GI             # 4 groups
NT = GI * MCHK // 128     # 9 tiles of [128, NVAR] per group
HC = NVAR // 2            # 288, matmul N-chunk (<=512 per PSUM bank)
SENT = 1024.0             # off-mask sentinel magnitude at t=0 (pre-decay)

# lq is 2*(2*atanh(r)); clip at 2*C where C = 2*atanh(1-1e-6)
_CLIP2 = float(4.0 * np.arctanh(np.float64(np.float32(1.0 - 1e-6))))


def _build(gate: float):
    nc = bacc.Bacc("TRN2", target_bir_lowering=False, debug=False)

    wg_d = nc.dram_tensor("wg", [BL * MCHK, NVAR], F16, kind="ExternalInput").ap()
    ms_d = nc.dram_tensor("ms", [BL * MCHK, NVAR], F16, kind="ExternalInput").ap()
    xs_d = nc.dram_tensor("xs", [BL, T * NVAR], F32, kind="ExternalInput").ap()
    xh_d = nc.dram_tensor("xh", [BL, NVAR], F16, kind="ExternalInput").ap()
    esel_d = nc.dram_tensor("esel", [128, NT * GI], F16, kind="ExternalInput").ap()
    eselt_d = nc.dram_tensor("eselt", [GI, NT * 128], F16, kind="ExternalInput").ap()
    negi_d = nc.dram_tensor("negi", [128, 128], F16, kind="ExternalInput").ap()
    posts_d = nc.dram_tensor("posts", [BL, T * NVAR], F32, kind="ExternalOutput").ap()

    one_m_g = float(1.0 - gate)
    posts_v = posts_d.rearrange("b (t c n) -> b t c n", t=T, c=2)

    with tile.TileContext(nc) as tc:
        with (
            tc.tile_pool(name="consts", bufs=1) as consts,
            tc.tile_pool(name="wg", bufs=2) as wg_pool,
            tc.tile_pool(name="mstate", bufs=2) as m_pool,
            tc.tile_pool(name="te", bufs=2) as te_pool,
            tc.tile_pool(name="pscan", bufs=2) as ps_pool,
            tc.tile_pool(name="a12", bufs=1) as a12_pool,
            tc.tile_pool(name="l12", bufs=1) as l12_pool,
            tc.tile_pool(name="lq", bufs=2) as lq_pool,
            tc.tile_pool(name="acur", bufs=1) as a_pool,
            tc.tile_pool(name="psum_v", bufs=2, space="PSUM") as psv_pool,
            tc.tile_pool(name="psum_post", bufs=2, space="PSUM") as psp_pool,
        ):
            esel = consts.tile([128, NT, GI], F16)
            nc.sync.dma_start(out=esel, in_=esel_d.rearrange("p (j g) -> p j g", g=GI))
            eselt = consts.tile([GI, NT, 128], F16)
            nc.sync.dma_start(
                out=eselt, in_=eselt_d.rearrange("g (j p) -> g j p", p=128)
            )
            negi = consts.tile([128, 128], F16)
            nc.sync.dma_start(out=negi, in_=negi_d)
            b38 = consts.tile([128, 1], F32)
            nc.vector.memset(b38, 1e-38)
            xsall = consts.tile([128, T, 2, HC], F32)
            for g in range(NG):
                nc.sync.dma_start(
                    out=xsall[32 * g : 32 * g + GI],
                    in_=xs_d[g * GI : (g + 1) * GI].rearrange(
                        "b (t c n) -> b t c n", t=T, c=2
                    ),
                )

            # tile pairs for pair-wide tail ops: (0,1) (2,3) (4,5) (6,7) (8,)
            pairs = [(0, 1), (2, 3), (4, 5), (6, 7), (8,)]

            def load_group(g):
                st = {"g": g, "k": g % 2}
                wg_g = wg_pool.tile([128, NT, NVAR], F16)
                nc.sync.dma_start(
                    out=wg_g,
                    in_=wg_d[g * NT * 128 : (g + 1) * NT * 128, :].rearrange(
                        "(j p) n -> p j n", p=128
                    ),
                )
                m_g = m_pool.tile([128, NT, NVAR], F16, name="m")
                nc.sync.dma_start(
                    out=m_g,
                    in_=ms_d[g * NT * 128 : (g + 1) * NT * 128, :].rearrange(
                        "(j p) n -> p j n", p=128
                    ),
                )
                a16 = a_pool.tile([GI, 2, HC], F16, tag=f"a16{g % 2}", name="a16")
                nc.sync.dma_start(
                    out=a16,
                    in_=xh_d[g * GI : (g + 1) * GI].rearrange("b (c n) -> b c n", c=2),
                )
                st["wg"] = wg_g
                st["m"] = m_g
                st["a16"] = a16
                st["xs"] = xsall[32 * g : 32 * g + GI]
                return st

            def phase_ab(st, t):
                """PE broadcast matmuls; ACT tanh straight from PSUM."""
                k = st["k"]
                te_all = te_pool.tile([128, NT, NVAR], F16, tag=f"te{k}", name="te")
                st["te"] = te_all
                for j in range(NT):
                    v_ps = psv_pool.tile([128, 2, 512], F32)
                    for c in range(2):
                        nc.tensor.matmul(
                            v_ps[:, c, :HC], eselt[:, j], st["a16"][:, c],
                            start=True, stop=False,
                        )
                        nc.tensor.matmul(
                            v_ps[:, c, :HC], negi,
                            st["m"][:, j, c * HC : (c + 1) * HC],
                            start=False, stop=True,
                        )
                    nc.scalar.activation(
                        te_all[:, j].rearrange("p (c n) -> p c n", c=2),
                        v_ps[:, :, :HC], ACT.Tanh, scale=0.5,
                    )

            def phase_cd(st, t):
                """Scans + squares (ln-table), then one giant Ln."""
                k = st["k"]
                te_all = st["te"]
                a12 = a12_pool.tile([128, NT, 2, NVAR], BF16, tag="a12", name="a12")
                for pj in pairs:
                    ps = ps_pool.tile([128, 2, HC], F16, tag=f"ps{k}", name="ps")
                    for jj, j in enumerate(pj):
                        nc.vector.tensor_tensor_scan(
                            out=ps[:, jj], data0=te_all[:, j, :HC],
                            data1=te_all[:, j, HC:],
                            initial=1.0, op0=ALU.mult, op1=ALU.mult,
                        )
                    for jj, j in enumerate(pj):
                        p_t = ps[:, jj, HC - 1 : HC]
                        nc.scalar.activation(
                            a12[:, j, 0], te_all[:, j], ACT.Square, bias=p_t, scale=1.0
                        )
                        nc.scalar.activation(
                            a12[:, j, 1], te_all[:, j], ACT.Square, bias=p_t, scale=-1.0
                        )
                l12 = l12_pool.tile([128, NT, 2, NVAR], F16, tag=f"l12{k}", name="l12")
                nc.scalar.activation(l12, a12, ACT.Ln, bias=b38)
                st["l12"] = l12

            def phase_f(st, t):
                """Pair-wide tail: lq, clip, wg-mult, gate update, posterior."""
                k = st["k"]
                g = st["g"]
                l12 = st["l12"]
                m_g, wg_g = st["m"], st["wg"]
                post_ps = psp_pool.tile([GI, 2, 512], F32)
                for pj in pairs:
                    w = len(pj)
                    j0 = pj[0]
                    lqf = lq_pool.tile([128, 2, NVAR], F16, tag=f"lq{k}",
                                       name="lq")[:, :w]
                    nc.vector.tensor_tensor(
                        out=lqf, in0=l12[:, j0 : j0 + w, 0],
                        in1=l12[:, j0 : j0 + w, 1], op=ALU.subtract,
                    )
                    nc.vector.tensor_scalar(
                        out=lqf, in0=lqf, scalar1=_CLIP2, scalar2=-_CLIP2,
                        op0=ALU.min, op1=ALU.max,
                    )
                    nc.gpsimd.tensor_tensor(out=lqf, in0=lqf,
                                            in1=wg_g[:, j0 : j0 + w], op=ALU.mult)
                    nc.vector.scalar_tensor_tensor(
                        out=m_g[:, j0 : j0 + w], in0=m_g[:, j0 : j0 + w],
                        scalar=one_m_g, in1=lqf, op0=ALU.mult, op1=ALU.add,
                    )
                    for j in pj:
                        for c in range(2):
                            nc.tensor.matmul(
                                post_ps[:, c, :HC],
                                esel[:, j],
                                m_g[:, j, c * HC : (c + 1) * HC],
                                start=(j == 0),
                                stop=(j == NT - 1),
                            )
                # posts_raw[g, :, t, :] = post (host corrects sentinel, adds x_t)
                p32 = a_pool.tile([GI, 2, HC], F32, tag=f"p32{k}", name="p32")
                nc.vector.tensor_copy(p32, post_ps[:, :, :HC])
                nc.sync.dma_start(out=posts_v[g * GI : (g + 1) * GI, t], in_=p32)
                if t + 1 < T:
                    a16 = a_pool.tile([GI, 2, HC], F16, tag=f"a16{k}", name="a16")
                    nc.vector.tensor_add(a16, post_ps[:, :, :HC], st["xs"][:, t + 1])
                    st["a16"] = a16

            for gpair in ((0, 1), (2, 3)):
                sts = [load_group(g) for g in gpair]
                for st in sts:
                    phase_ab(st, 0)
                for t in range(T):
                    for st in sts:
                        phase_cd(st, t)
                    for st in sts:
                        phase_f(st, t)
                        if t + 1 < T:
                            phase_ab(st, t + 1)
    nc.compile()
    return nc


_CACHE = {}


def _get_nc(gate: float):
    key = round(gate, 12)
    if key not in _CACHE:
        _CACHE[key] = _build(gate)
    return _CACHE[key]


def _host_prep(inputs, H, sigma2, input_ponderation, w_cv, gate_logit):
    f32 = np.float32
    f16 = np.float16
    gate = float(1.0 / (1.0 + np.exp(-np.float64(gate_logit))))

    llrs = (f32(-4.0) * inputs / sigma2).astype(f32)
    norm_llrs = llrs / np.mean(np.abs(llrs), axis=-1, keepdims=True, dtype=f32)
    xs = (norm_llrs[:, None, :] * input_ponderation[None, :, :]).astype(f32)  # [B,T,N]
    xh0 = xs[:, 0].astype(f16)

    # sentinel corrections: posterior of iter t-1 is short by
    # SENT*2^{-t} per off-mask row of each column
    offc = (f32(MCHK) - H.sum(axis=1)).astype(f32)                # [B, N]
    xs_dev = xs.copy()
    for t in range(1, T):
        xs_dev[:, t] += f32(SENT * (2.0 ** (-t))) * offc
    post_corr = np.stack(
        [f32(SENT * (2.0 ** (-(t + 1)))) * offc for t in range(T)], axis=1
    )  # [B, T, N]

    # lq = 2*(2*atanh(r)); fold the 0.5 and the gate into the edge weights
    wgp = (f32(0.5 * gate) * w_cv[None, :, :] * H.astype(f32)).astype(f16)  # [B,M,N]
    msinit = (-f32(SENT) * (f32(1.0) - H.astype(f32))).astype(f16)

    # selector constants (same for every core)
    rows = np.arange(GI * MCHK)
    esel = np.zeros((128, NT, GI), f16)
    eselt = np.zeros((GI, NT, 128), f16)
    for j in range(NT):
        for p in range(128):
            k = int(rows[j * 128 + p] // MCHK)
            esel[p, j, k] = 1.0
            eselt[k, j, p] = 1.0
    negi = (-np.eye(128)).astype(f16)

    in_maps = []
    for c in range(NCORES):
        sl = slice(c * BL, (c + 1) * BL)
        in_maps.append(
            {
                "wg": np.ascontiguousarray(wgp[sl].reshape(BL * MCHK, NVAR)),
                "ms": np.ascontiguousarray(msinit[sl].reshape(BL * MCHK, NVAR)),
                "xs": np.ascontiguousarray(xs_dev[sl].reshape(BL, T * NVAR)),
                "xh": np.ascontiguousarray(xh0[sl]),
                "esel": np.ascontiguousarray(esel.reshape(128, NT * GI)),
                "eselt": np.ascontiguousarray(eselt.reshape(GI, NT * 128)),
                "negi": negi,
            }
        )
    return gate, norm_llrs, xs, post_corr, in_maps


def _host_post(posts_raw, xs, norm_llrs, post_corr, out_ponderation,
               skip_ponderation):
    f32 = np.float32
    posts = (posts_raw + post_corr + xs).astype(f32)  # un-poison + add x_t
    norm_out = posts / np.mean(np.abs(posts), axis=-1, keepdims=True, dtype=f32)
    pooled = np.mean(out_ponderation[None] * norm_out, axis=-2, dtype=f32)
    out = (pooled + skip_ponderation * norm_llrs).astype(f32)
    return (1.0 / (1.0 + np.exp(out[:, :KINFO], dtype=f32))).astype(f32)


def run(trace=False, **inputs):
    inputs = {k: np.asarray(v) for k, v in inputs.items()}
    gate, norm_llrs, xs, post_corr, in_maps = _host_prep(
        inputs["inputs"],
        inputs["H"],
        inputs["sigma2"],
        inputs["input_ponderation"],
        inputs["w_cv"],
        inputs["gate_logit"],
    )
    nc = _get_nc(gate)
    res = run_bass_kernel_spmd(
        nc, in_maps, core_ids=list(range(NCORES)), trace=trace
    )
    posts_raw = np.concatenate(
        [r["posts"].astype(np.float32).reshape(BL, T, NVAR) for r in res.results],
        axis=0,
    )
    out = _host_post(
        posts_raw, xs, norm_llrs, post_corr,
        inputs["out_ponderation"], inputs["skip_ponderation"],
    )
    return out, res


def kernel(**inputs) -> np.ndarray:
    out, _ = run(trace=False, **inputs)
    return out
